# revision 4
# baseline (speedup 1.0000x reference)
"""v33: v32 + half-width PV for fullmask blocks (their pt half is exactly zero). v28 + fullmask score blocks computed half-width (the -1e30 add saturates the stale half to exp->0). v19 + masked diagonal pair processed first within each head (shortens the head-tail dependency chain).  Base: v12 + sums/recipT merged into the PV bank, ot ring doubled.  Base: v12 = software-pipelined: attention for chunk N runs after projections of chunk N+1, so attention latency bubbles fill with dense proj matmuls.  Base: v11 = v10 + DMA issue-order fixes (prefetch next chunk x, deferred out_slice copies, split weight loads).  Base: v10 = v9 + cheap softmax sums (P-stationary 1-row matmuls) with deferred normalize chains.

Projections/RoPE for each 512-column s-chunk are immediately followed by
attention + wo for the two supertiles that chunk unlocks (chunk order
0,1,4,5,2,3 makes K/V blocks available exactly when needed).  Partials
stream out through 5 staggered bf16 ReduceScatter groups.  All DRAM
traffic is bf16; PSUM stays f32.

PSUM budget (8 banks): tag "big" x3 (proj accumulation waves of <=3 heads
+ wo pp ring), "sc" x3 (score pairs), "ot" x1 (PV), "sm" x1 (softmax sums
+ V-transpose staging).

Engine split: PE matmuls; Act = exp only; DVE = rope muls/shuffle,
reciprocal, at-mul, half the wo evacs; Pool(gpsimd) = psum evacs, mask
adds, broadcasts, other half of wo evacs.
"""
import numpy as np
import ml_dtypes

import concourse.bass as bass
import concourse.mybir as mybir
import concourse.tile as tile
from concourse import bacc
from concourse.bass_utils import run_bass_kernel_spmd
from concourse.masks import make_identity

F32 = mybir.dt.float32
F32R = mybir.dt.float32r
BF16 = mybir.dt.bfloat16

N_CORES = 8
S = 3072
D = 4096
HD = 128
HQ = 4
QO = HQ * HD
KC = 2048
NEG = -1.0e30
EXP_BIAS = -20.0
N_KT = D // 128

SC_ORDER = (0, 1, 4, 5, 2, 3)      # s-chunk processing order
# each chunk unlocks the two supertiles covering its 512 queries
SC_STS = {0: (0, 1), 1: (2, 3), 4: (8, 9), 5: (10, 11), 2: (4, 5), 3: (6, 7)}
RS_GROUPS = [(0, 1, 2, 3), (8, 9, 10, 11), (4, 5), (6,), (7,)]

_NC_CACHE = {}


def _nvis(st):
    if st < 4:
        return 2 * st + 2
    if st < 8:
        return 8
    return 2 * (st - 8) + 10


def _masks(st):
    if 4 <= st < 8:
        return []
    kb = 2 * st if st < 4 else 2 * st - 8
    return [(0, kb, 0), (0, kb + 1, 1), (1, kb + 1, 0)]


def build_kernel():
    nc = bacc.Bacc("TRN2", target_bir_lowering=False, debug=False,
                   num_devices=N_CORES)

    xT = nc.dram_tensor("xT", [D, S], BF16, kind="ExternalInput").ap()
    wqT = nc.dram_tensor("wqT", [D, QO], BF16, kind="ExternalInput").ap()
    wkT = nc.dram_tensor("wkT", [D, HD], BF16, kind="ExternalInput").ap()
    wvT = nc.dram_tensor("wvT", [D, HD], BF16, kind="ExternalInput").ap()
    woT = nc.dram_tensor("woT", [QO, D], BF16, kind="ExternalInput").ap()
    csq = nc.dram_tensor("csq", [HD, 2, S], BF16, kind="ExternalInput").ap()
    out_slice = nc.dram_tensor("out_slice", [S // N_CORES, D], BF16,
                               kind="ExternalOutput").ap()

    partials = [nc.dram_tensor(f"partial{g}", [256 * len(sts), D], BF16).ap()
                for g, sts in enumerate(RS_GROUPS)]
    rs_outs = [nc.dram_tensor(f"rs_out{g}", [32 * len(sts), D], BF16).ap()
               for g, sts in enumerate(RS_GROUPS)]

    shuf_mask = [j ^ 1 for j in range(32)]

    st_group = {}
    for g, sts in enumerate(RS_GROUPS):
        for i, st in enumerate(sts):
            st_group[st] = (g, i)

    with tile.TileContext(nc) as tc:
        with tc.tile_pool(name="glob", bufs=1) as gpool, \
             tc.tile_pool(name="wts", bufs=1) as wpool, \
             tc.tile_pool(name="xp", bufs=9) as xpool, \
             tc.tile_pool(name="csp", bufs=2) as cspool, \
             tc.tile_pool(name="rtmp", bufs=2) as tpool, \
             tc.tile_pool(name="ptp", bufs=2) as ppool, \
             tc.tile_pool(name="smal", bufs=4) as spool, \
             tc.tile_pool(name="atp", bufs=2) as atpool, \
             tc.tile_pool(name="pop", bufs=2) as popool, \
             tc.tile_pool(name="psum", bufs=1, space="PSUM") as psp:
            ident = gpool.tile([128, 128], F32)
            make_identity(nc, ident[:])
            identb = gpool.tile([128, 128], BF16)
            nc.vector.tensor_copy(identb[:], ident[:])
            identr = gpool.tile([128, 128], F32R)
            nc.vector.tensor_copy(identr[:], ident[:])
            # transposed causal mask: scT[k, q] visible iff q >= k
            utmask = gpool.tile([128, 128], F32)
            nc.gpsimd.memset(utmask[:], 0.0)
            nc.gpsimd.affine_select(
                out=utmask[:], in_=utmask[:],
                compare_op=mybir.AluOpType.is_ge, fill=NEG,
                base=0, pattern=[[1, 128]], channel_multiplier=-1)
            ones_b = gpool.tile([128, 1], BF16)
            nc.gpsimd.memset(ones_b[:], 1.0)
            fullmask = gpool.tile([128, 128], F32)
            nc.gpsimd.memset(fullmask[:], NEG)
            bias_t = gpool.tile([128, 1], F32)
            nc.gpsimd.memset(bias_t[:], EXP_BIAS)
            kt_res = gpool.tile([128, KC], BF16)
            v_res = gpool.tile([128, KC // 128, HD], BF16)
            qt_sb = gpool.tile([128, HQ, S], BF16)

            # weights: chunked loads so first matmuls start early
            wk_sb = wpool.tile([128, N_KT, HD], BF16)
            nc.sync.dma_start(
                wk_sb[:, 0:8, :],
                wkT.rearrange("(kt p) o -> p kt o", p=128)[:, 0:8, :])
            wv_sb = wpool.tile([128, N_KT, HD], BF16)
            nc.sync.dma_start(
                wv_sb[:, 0:8, :],
                wvT.rearrange("(kt p) o -> p kt o", p=128)[:, 0:8, :])
            wq_sb = wpool.tile([128, N_KT, QO], BF16)
            wo_sb = wpool.tile([128, HQ, D], BF16)

            done_sts = set()
            fired = set()
            pending_norm = []
            deferred_out = []
            def proj_part(ci, sc, cs_sb, pf_xts):
                cached = sc in (0, 1, 4, 5)
                scol = slice(sc * 512, (sc + 1) * 512)
                cos_sb = cs_sb[:, 0]
                sin_sb = cs_sb[:, 1]


                # ---- projections, in <=3-head PSUM waves ----
                if cached:
                    waves = [[4, 5], [0, 1], [2, 3]]
                else:
                    waves = [[0, 1], [2, 3]]
                if ci == 0:
                    xts = []
                    for ktg in range(N_KT // 4):
                        xt = xpool.tile([128, 4, 512], BF16, tag="xt")
                        nc.sync.dma_start(
                            xt[:],
                            xT.rearrange("(kt p) s -> p kt s", p=128)[
                                :, ktg * 4:(ktg + 1) * 4, scol])
                        xts.append(xt)
                        if ktg in (0, 2, 4, 6):
                            kg = ktg // 2
                            nc.sync.dma_start(
                                wq_sb[:, kg * 8:(kg + 1) * 8, :],
                                wqT.rearrange("(kt p) o -> p kt o", p=128)[
                                    :, kg * 8:(kg + 1) * 8, :])
                        if ktg == 1:
                            nc.sync.dma_start(
                                wk_sb[:, 8:32, :],
                                wkT.rearrange("(kt p) o -> p kt o", p=128)[
                                    :, 8:32, :])
                            nc.sync.dma_start(
                                wv_sb[:, 8:32, :],
                                wvT.rearrange("(kt p) o -> p kt o", p=128)[
                                    :, 8:32, :])
                    for h in range(HQ):
                        nc.sync.dma_start(
                            wo_sb[:, h, :],
                            woT.rearrange("(h p) o -> p h o", p=128)[:, h, :])
                else:
                    xts = pf_xts
                rope_tiles = {}
                for wi, wave in enumerate(waves):
                    psums = {}
                    for i in wave:
                        # K/V wave allocates from the "sc" ring: those slots
                        # free after the last exp, not after wo evacs, so the
                        # next chunk's projections start without stalling
                        tag = "sc" if (cached and wi == 0) else "big"
                        psums[i] = psp.tile([128, 512], F32, tag=tag,
                                            bufs=3, name=f"proj{i}")
                    for ktg in range(N_KT // 4):
                        for ktl in range(4):
                            kt = ktg * 4 + ktl
                            for i in wave:
                                if i < HQ:
                                    w = wq_sb[:, kt, i * 128:(i + 1) * 128]
                                elif i == HQ:
                                    w = wk_sb[:, kt, :]
                                else:
                                    w = wv_sb[:, kt, :]
                                nc.tensor.matmul(psums[i][:], w,
                                                 xts[ktg][:, ktl],
                                                 start=(kt == 0),
                                                 stop=(kt == N_KT - 1))
                    # rope / V evacuation for this wave
                    for i in wave:
                        if i < 5:
                            ps = tpool.tile([128, 512], BF16, tag="evc")
                            nc.scalar.copy(out=ps[:], in_=psums[i][:])
                            shuf = tpool.tile([128, 512], BF16, tag="shuf")
                            nc.vector.stream_shuffle(shuf[:], ps[:],
                                                     shuf_mask)
                            t1 = tpool.tile([128, 512], BF16, tag="t1")
                            nc.gpsimd.tensor_mul(t1[:], ps[:], cos_sb)
                            t2 = tpool.tile([128, 512], BF16, tag="t2")
                            nc.gpsimd.tensor_mul(t2[:], shuf[:], sin_sb)
                            if i < HQ:
                                nc.gpsimd.tensor_add(qt_sb[:, i, scol],
                                                     t1[:], t2[:])
                            else:
                                kcol = sc * 512 if sc < 2 else (sc - 2) * 512
                                nc.gpsimd.tensor_add(
                                    kt_res[:, kcol:kcol + 512], t1[:], t2[:])
                        else:
                            vt = tpool.tile([128, 512], BF16, tag="vt")
                            nc.scalar.copy(out=vt[:], in_=psums[i][:])
                            vps = psp.tile([128, 512], BF16, tag="ot",
                                           bufs=2, name="vps",
                                           padded_shape=[128, 1024])
                            for j in range(4):
                                nc.tensor.transpose(
                                    vps[:, j * 128:(j + 1) * 128],
                                    vt[:, j * 128:(j + 1) * 128], identb[:])
                            vb = sc * 4 if sc < 2 else (sc - 2) * 4
                            nc.vector.tensor_copy(
                                out=v_res[:, vb:vb + 4, :], in_=vps[:])

                # prefetch next chunk's x tiles (+cos/sin) now, ahead of
                # later store DMAs, to dodge SP head-of-line blocking
                nxt = None
                if ci + 1 < len(SC_ORDER):
                    nsc = SC_ORDER[ci + 1]
                    nscol = slice(nsc * 512, (nsc + 1) * 512)
                    ncs = cspool.tile([128, 2, 512], BF16, tag="cs")
                    nc.sync.dma_start(ncs[:], csq[:, :, nscol])
                    pf = []
                    for ktg in range(N_KT // 4):
                        xt = xpool.tile([128, 4, 512], BF16, tag="xt")
                        nc.sync.dma_start(
                            xt[:],
                            xT.rearrange("(kt p) s -> p kt s", p=128)[
                                :, ktg * 4:(ktg + 1) * 4, nscol])
                        pf.append(xt)
                    nxt = (ncs, pf)
                return nxt

            def attn_part(ci, sc):
                # ---- attention for the two supertiles this chunk unlocks,
                # then wo for both (wo delayed so normalize chains hide) ----
                sts = SC_STS[sc]
                st_at = {}
                for st in sts:
                    nvis = _nvis(st)
                    qcol = slice(st * 256, (st + 1) * 256)
                    at_tiles = []
                    for h in range(HQ):
                        qt_slice = qt_sb[:, h, qcol]
                        pt_sb = ppool.tile([128, 16, 256], BF16, tag="pt")
                        # one bank per head: PV cols [0:256], sums [256:258],
                        # transposed reciprocals (bf16) in cols [384:512]
                        ot_ps = psp.tile([128, 512], F32, tag="ot",
                                         bufs=2, name="ot")
                        masks = _masks(st)
                        # process the masked diagonal pair first so its extra
                        # DVE mask-add latency hides behind later pairs
                        pair_order = list(range(0, nvis, 2))
                        if masks:
                            pair_order = [pair_order[-1]] + pair_order[:-1]
                        fm_kb = {mkb for (_, mkb, kind) in masks
                                 if kind == 1}
                        for pi, kb0 in enumerate(pair_order):
                            scT = psp.tile([128, 2, 256], F32, tag="sc",
                                           bufs=3, name="scT")
                            for j in range(2):
                                kb = kb0 + j
                                if kb in fm_kb:
                                    # q-half 0 is fully masked; its -1e30
                                    # add below saturates the stale psum,
                                    # so only compute q-half 1
                                    nc.tensor.matmul(
                                        scT[:, j, 128:256],
                                        kt_res[:, kb * 128:(kb + 1) * 128],
                                        qt_slice[:, 128:256],
                                        start=True, stop=True)
                                    continue_scores = True
                                else:
                                    nc.tensor.matmul(
                                        scT[:, j],
                                        kt_res[:, kb * 128:(kb + 1) * 128],
                                        qt_slice, start=True, stop=True)
                                for (mqi, mkb, kind) in masks:
                                    if mkb != kb:
                                        continue
                                    m = utmask if kind == 0 else fullmask
                                    nc.vector.tensor_add(
                                        scT[:, j, mqi * 128:(mqi + 1) * 128],
                                        scT[:, j, mqi * 128:(mqi + 1) * 128],
                                        m[:])
                            nc.scalar.activation(
                                pt_sb[:, kb0:kb0 + 2, :], scT[:],
                                mybir.ActivationFunctionType.Exp,
                                bias=bias_t[:], scale=1.0)
                            if pi == 0 and pending_norm:
                                pending_norm.pop(0)()
                            for j in range(2):
                                kb = kb0 + j
                                if kb in fm_kb:
                                    # dead pt half is exactly zero; only
                                    # accumulate the live q-half (never the
                                    # chain start: j==0 of pair 0 is unmasked)
                                    nc.tensor.matmul(
                                        ot_ps[:, 128:256], v_res[:, kb, :],
                                        pt_sb[:, kb, 128:256],
                                        start=False,
                                        stop=(pi == len(pair_order) - 1
                                              and j == 1))
                                else:
                                    nc.tensor.matmul(
                                        ot_ps[:, 0:256], v_res[:, kb, :],
                                        pt_sb[:, kb, :],
                                        start=(pi == 0 and j == 0),
                                        stop=(pi == len(pair_order) - 1
                                              and j == 1))
                        # softmax sums: P-stationary 1-row matmuls, two
                        # sequential chains (one per q-half) in the PV bank
                        for qh in range(2):
                            for kb in range(nvis):
                                nc.tensor.matmul(
                                    ot_ps[:, 256 + qh:257 + qh],
                                    pt_sb[:, kb, qh * 128:(qh + 1) * 128],
                                    ones_b[:],
                                    start=(kb == 0), stop=(kb == nvis - 1))
                        recip_sb = spool.tile([128, 2], BF16, tag="rcp",
                                              name="recip")
                        with nc.allow_low_precision(reason="bf16 recip"):
                            nc.vector.reciprocal(recip_sb[:],
                                                 ot_ps[:, 256:258])
                        at_sb = atpool.tile([128, 256], BF16, tag=f"at{h}",
                                            name="at")

                        def norm_chain(recip_sb=recip_sb, ot_ps=ot_ps,
                                       at_sb=at_sb):
                            rcT = ot_ps[0:1, 384:512].bitcast(BF16)
                            nc.tensor.transpose(rcT[0:1, 0:128],
                                                recip_sb[:, 0:1], identb[:])
                            nc.tensor.transpose(rcT[0:1, 128:256],
                                                recip_sb[:, 1:2], identb[:])
                            rc_sb = spool.tile([1, 256], BF16, tag="rsb",
                                               name="rc_sb")
                            nc.vector.tensor_copy(out=rc_sb[:], in_=rcT[:])
                            bc = spool.tile([128, 256], BF16, tag="bc",
                                            name="bc")
                            nc.gpsimd.partition_broadcast(bc[:], rc_sb[:])
                            nc.vector.tensor_mul(at_sb[:], ot_ps[:, 0:256],
                                                 bc[:])

                        pending_norm.append(norm_chain)
                        at_tiles.append(at_sb)
                    st_at[st] = at_tiles

                for st in sts:
                    at_tiles = st_at[st]
                    g, gi = st_group[st]
                    while pending_norm:
                        pending_norm.pop(0)()
                    for j in range(2):
                        po_sb = popool.tile([128, D], BF16, tag="po")
                        for oc in range(8):
                            pp = psp.tile([128, 512], F32, tag="big",
                                          bufs=3, name="pp")
                            for h in range(HQ):
                                nc.tensor.matmul(
                                    pp[:],
                                    at_tiles[h][:, j * 128:(j + 1) * 128],
                                    wo_sb[:, h, oc * 512:(oc + 1) * 512],
                                    start=(h == 0), stop=(h == HQ - 1))
                            if oc % 2 == 0:
                                nc.vector.tensor_copy(
                                    out=po_sb[:, oc * 512:(oc + 1) * 512],
                                    in_=pp[:])
                            else:
                                nc.scalar.copy(
                                    out=po_sb[:, oc * 512:(oc + 1) * 512],
                                    in_=pp[:])
                        row = (2 * gi + j) * 128
                        nc.sync.dma_start(partials[g][row:row + 128, :],
                                          po_sb[:])

                    done_sts.add(st)
                    for g2, gsts in enumerate(RS_GROUPS):
                        if g2 in fired:
                            continue
                        if all(s in done_sts for s in gsts):
                            fired.add(g2)
                            nrows = 32 * len(gsts)
                            off = sum(32 * len(RS_GROUPS[gg])
                                      for gg in range(g2))
                            nc.gpsimd.collective_compute(
                                "ReduceScatter", mybir.AluOpType.add,
                                replica_groups=[list(range(N_CORES))],
                                ins=[partials[g2]], outs=[rs_outs[g2]])
                            deferred_out.append((off, nrows, g2))

            cs0 = cspool.tile([128, 2, 512], BF16, tag="cs", name="cs0")
            nc.sync.dma_start(cs0[:], csq[:, :, 0:512])
            carry = (cs0, None)
            for ci, sc in enumerate(SC_ORDER):
                carry = proj_part(ci, sc, carry[0], carry[1])
                if ci >= 1:
                    attn_part(ci - 1, SC_ORDER[ci - 1])
            attn_part(len(SC_ORDER) - 1, SC_ORDER[-1])

            for (off, nrows, g2) in deferred_out:
                nc.sync.dma_start(out_slice[off:off + nrows, :],
                                  rs_outs[g2])

    nc.compile()
    return nc


def _host_prep(x, wq, wk, wv, wo, freqs):
    bf = ml_dtypes.bfloat16
    xT = np.ascontiguousarray(x[0].T).astype(bf)
    scale = np.float32(HD ** -0.25)
    cos = (np.cos(freqs) * scale).astype(np.float32).T
    sin = (np.sin(freqs) * scale).astype(np.float32).T
    csq = np.empty((HD, 2, S), np.float32)
    csq[0::2, 0] = cos
    csq[1::2, 0] = cos
    csq[0::2, 1] = -sin
    csq[1::2, 1] = sin
    csq = csq.astype(bf)
    in_maps = []
    for c in range(N_CORES):
        in_maps.append({
            "xT": xT,
            "csq": csq,
            "wqT": np.ascontiguousarray(wq[c * QO:(c + 1) * QO].T).astype(bf),
            "wkT": np.ascontiguousarray(wk[c * HD:(c + 1) * HD].T).astype(bf),
            "wvT": np.ascontiguousarray(wv[c * HD:(c + 1) * HD].T).astype(bf),
            "woT": np.ascontiguousarray(wo[:, c * QO:(c + 1) * QO].T).astype(bf),
        })
    return in_maps


def kernel(x, wq, wk, wv, wo, freqs, start_pos):
    assert int(start_pos) == 0
    x = np.asarray(x, np.float32)
    wq = np.asarray(wq, np.float32)
    wk = np.asarray(wk, np.float32)
    wv = np.asarray(wv, np.float32)
    wo = np.asarray(wo, np.float32)
    freqs = np.asarray(freqs, np.float32)

    if "nc" not in _NC_CACHE:
        _NC_CACHE["nc"] = build_kernel()
    nc = _NC_CACHE["nc"]

    in_maps = _host_prep(x, wq, wk, wv, wo, freqs)
    res = run_bass_kernel_spmd(nc, in_maps, list(range(N_CORES)))
    out = np.empty((S, D), np.float32)
    for c in range(N_CORES):
        piece = np.asarray(res.results[c]["out_slice"]).astype(np.float32)
        off = 0
        for g, sts in enumerate(RS_GROUPS):
            r8 = 32 * len(sts)
            sub = piece[off:off + r8]
            rows = c * r8 + np.arange(r8)
            grows = 256 * np.asarray(sts)[rows // 256] + rows % 256
            out[grows] = sub
            off += r8
    return out.reshape(1, S, D)
